# revision 27
# baseline (speedup 1.0000x reference)
"""Trainium2 Bass kernel for MeshDihedralAngleLoss.

Reference computation (per batch b, per edge e with ep = edge_points[b,e] =
[v0, v1, a, b]):
    na = normalize(cross(verts[a]-verts[v0], verts[v1]-verts[v0]))
    nb = normalize(cross(verts[b]-verts[v1], verts[v0]-verts[v1]))
    angle = pi - arccos(clip(dot(na, nb), +-(1-1e-7)))
computed for vert1 and vert2; loss = mean_b mean_e (angle1-angle2)^2.

Algebra: with ea = p2-p0, eb = p1-p0, d3 = p3-p1, nb = cross(d3, -eb) = -m,
    angle = pi - arccos(-u) = arccos(u),  u = dot(na, m)/(|na||m|)
and with q = dot(na, m), w = |na||m|:
    arccos(u) = 2*atan(sqrt((w - q)/(w + q)))
              = 2*atan(exp(0.5*(ln(w - q) - ln(w + q))))     [division-free]
so angle1 - angle2 = 2*(atan(t1) - atan(t2)) and the host applies the *4
factor on the squared sums plus the global mean (the only cross-core step).
w -+ q are clamped to [1e-4, inf): this covers f32/f16 rounding pushing them
negative AND the u-clip of the reference (different floor, same measure-zero
edges), and keeps Ln's input strictly positive with no bias const needed.

Sharding: pure data parallel, core b <- mesh b (B == 8 == n_cores).

Host marshaling: the indexed gather is pure data movement, so it is done
host-side with numpy fancy indexing (same class of marshaling as index
pre-tiling): each core receives its edges' vertex coordinates pre-gathered
(fp16) into the exact plane-major SBUF layout
    pg[t, p, ((m*3 + c)*4 + j)*F + w] = verts_m[ep[e, j], c],
    e = (t*P + p)*F + w
(m = mesh 0/1, c = xyz, j = vertex slot 0..3).  The device then streams
sequential DRAM at full DMA bandwidth -- no per-edge descriptors.

Engine split (software-pipelined by one chunk so DVE never waits on ACT):
  DVE:  edge vectors, cross products, dot products (fp16, 2x mode) + glue
  ACT:  squares (split na/mm halves), sqrt, ln/exp (the division), arctan,
        final sum-accumulate
The last chunk's geometry takes the previous tail's stages as hooks placed
so its ACT ops interleave between the square halves on the ACT queue, and
the final two tails are stage-interleaved -- the end-of-program ACT chains
overlap remaining DVE work.
"""

import numpy as np

import concourse.bass as bass
import concourse.mybir as mybir
from concourse.tile import TileContext
from concourse.bass_utils import run_bass_kernel_spmd

dt = mybir.dt
Alu = mybir.AluOpType
AF = mybir.ActivationFunctionType

B, N, E = 8, 100000, 300000
P = 128
F = 586            # edges per partition per chunk
T = 4              # chunks; P*F*T = 300032 >= E (32 zero-padded edges)
EPAD = P * F * T
F2 = F // 2        # width of the half-tails pipelined at the kernel end
CLAMP = 1e-4       # fp16-normal floor for w -+ q (see module docstring)

_CACHE: dict = {}


def _build_program() -> bass.Bass:
    nc = bass.Bass(trn_type="TRN2")
    f32 = dt.float32
    f16 = dt.float16
    pg = nc.dram_tensor("pg", [T, P, 24 * F], f16, kind="ExternalInput")
    out = nc.dram_tensor("out", [P, T + 1], f32, kind="ExternalOutput")

    with TileContext(nc) as tc:
        with (
            tc.tile_pool(name="accp", bufs=1) as accp,
            tc.tile_pool(name="iop", bufs=2) as iop,
            tc.tile_pool(name="wkp", bufs=1) as wkp,
            tc.tile_pool(name="pip", bufs=2) as pip,
            tc.tile_pool(name="smp", bufs=1) as smp,
        ):
            acc = accp.tile([P, T + 1], f32)

            def geo(t, hooks=()):
                """DVE geometry for chunk t; returns pq = [na*mm | na^2|mm^2].
                hooks: up to 4 callables emitted between geometry phases (the
                last chunk interleaves the previous tail's stages here so its
                ACT ops slot between this chunk's square halves)."""
                hooks = list(hooks) + [None] * 4
                gall = iop.tile([P, 24 * F], f16, tag="gall")
                if t == 0:
                    # split the cold-start DMA, first slice = exactly the
                    # columns the first subtraction reads (mesh-0 planes y,z;
                    # Tile deps are per-slice and DMA queues drain in issue
                    # order, so compute starts ~1/3 into the transfer)
                    nc.sync.dma_start(
                        out=gall[:, 4 * F : 12 * F], in_=pg[t, :, 4 * F : 12 * F]
                    )
                    nc.sync.dma_start(out=gall[:, 0 : 4 * F], in_=pg[t, :, 0 : 4 * F])
                    nc.sync.dma_start(out=gall[:, 12 * F :], in_=pg[t, :, 12 * F :])
                else:
                    nc.sync.dma_start(out=gall[:], in_=pg[t, :, :])
                # planes: [m:2][c:3][j:4][w:F]
                gv = gall[:].rearrange("p (m c j w) -> p m c j w", m=2, c=3, j=4)

                # Edge vectors in 4-plane buffers (y,z,x,y per mesh) -- only
                # the planes the cross products read; component rotation is a
                # plain +-F offset shift.
                ea4 = wkp.tile([P, 8 * F], f16, tag="ea4")
                eb4 = wkp.tile([P, 8 * F], f16, tag="eb4")
                d34 = wkp.tile([P, 8 * F], f16, tag="d34")
                mesh_splits = ((0, 2),) if t != 0 else ((0, 1), (1, 2))
                for m0, m1 in mesh_splits:
                    for buf, js, jb in ((ea4, 2, 0), (eb4, 1, 0), (d34, 3, 1)):
                        bv = buf[:].rearrange("p (m k w) -> p m k w", m=2, k=4)
                        nc.vector.tensor_tensor(
                            out=bv[:, m0:m1, 0:2, :],
                            in0=gv[:, m0:m1, 1:3, js, :],
                            in1=gv[:, m0:m1, 1:3, jb, :],
                            op=Alu.subtract,
                        )
                        nc.vector.tensor_tensor(
                            out=bv[:, m0:m1, 2:4, :],
                            in0=gv[:, m0:m1, 0:2, js, :],
                            in1=gv[:, m0:m1, 0:2, jb, :],
                            op=Alu.subtract,
                        )
                if hooks[0]:
                    hooks[0]()

                # na = ea x eb ; mm = d3 x eb  (plane-major per mesh), packed
                # namm = [na_m0 | na_m1 | mm_m0 | mm_m1] x 3F
                namm = pip.tile([P, 12 * F], f16, tag="namm")
                pq = pip.tile([P, 18 * F], f16, tag="pq")
                tmp6 = wkp.tile([P, 6 * F], f16, tag="t6")
                t6v = tmp6[:].rearrange("p (m k w) -> p m k w", m=2, k=3)
                b4 = eb4[:].rearrange("p (m k w) -> p m k w", m=2, k=4)
                for half, (dst0, av) in enumerate(((0, ea4), (6 * F, d34))):
                    dstv = namm[:, dst0 : dst0 + 6 * F].rearrange(
                        "p (m k w) -> p m k w", m=2, k=3
                    )
                    a4 = av[:].rearrange("p (m k w) -> p m k w", m=2, k=4)
                    nc.vector.tensor_tensor(
                        out=dstv, in0=a4[:, :, 0:3, :], in1=b4[:, :, 1:4, :],
                        op=Alu.mult,
                    )
                    nc.vector.tensor_tensor(
                        out=t6v, in0=a4[:, :, 1:4, :], in1=b4[:, :, 0:3, :],
                        op=Alu.mult,
                    )
                    nc.vector.tensor_tensor(
                        out=dstv, in0=dstv, in1=t6v, op=Alu.subtract
                    )
                    # this half's squares on ACT, overlapping the next DVE work
                    nc.scalar.activation(
                        out=pq[:, 6 * F + dst0 : 12 * F + dst0],
                        in_=namm[:, dst0 : dst0 + 6 * F], func=AF.Square,
                    )
                    if hooks[1 + half]:
                        hooks[1 + half]()

                nc.vector.tensor_tensor(
                    out=pq[:, 0 : 6 * F], in0=namm[:, 0 : 6 * F],
                    in1=namm[:, 6 * F : 12 * F], op=Alu.mult,
                )
                if hooks[3]:
                    hooks[3](pq)
                return pq

            def tail_stages(col, pq, h=None):
                """Chunk tail as fine stages (DVE ops + the ACT hop they
                feed), emitted after the NEXT chunk's geometry.  h=0/1
                processes one edge-half (width F2) -- the kernel-end tails
                are split this way and interleaved so their ACT chains
                pipeline against the sibling half's DVE work.  Half 0 reuses
                the full-size tile tags (sliced); half 1 gets compact tags."""
                st = {}
                FW = F if h is None else F2   # logical width
                aw = F2 if h == 1 else F      # allocation width (tag sizing)
                sx = "1" if h == 1 else ""
                w0 = F2 if h == 1 else 0      # pq w-offset

                def s1():
                    # plane sums: [q | na2 | m2] for both meshes in one pair
                    pqv = pq[:].rearrange("p (g k w) -> p g k w", g=6, k=3)
                    qnm = smp.tile([P, 6 * aw], f16, tag="qnm" + sx)
                    qv = qnm[:, 0 : 6 * FW].rearrange("p (g w) -> p g w", g=6)
                    nc.vector.tensor_tensor(
                        out=qv, in0=pqv[:, :, 0, w0 : w0 + FW],
                        in1=pqv[:, :, 1, w0 : w0 + FW], op=Alu.add,
                    )
                    nc.vector.tensor_tensor(
                        out=qv, in0=qv, in1=pqv[:, :, 2, w0 : w0 + FW], op=Alu.add
                    )
                    st["qnm"] = qnm
                    # sqrt of the norms (ACT) queued right behind the sums
                    sn = smp.tile([P, 4 * aw], f16, tag="sn" + sx)
                    nc.scalar.activation(
                        out=sn[:, 0 : 4 * FW], in_=qnm[:, 2 * FW : 6 * FW],
                        func=AF.Sqrt,
                    )
                    st["sn"] = sn

                def s2():
                    # w = |na||m| = sqrt(na2)*sqrt(m2), fp16 2x
                    sn = st["sn"]
                    w01 = smp.tile([P, 2 * aw], f16, tag="w01" + sx)
                    nc.vector.tensor_tensor(
                        out=w01[:, 0 : 2 * FW], in0=sn[:, 0 : 2 * FW],
                        in1=sn[:, 2 * FW : 4 * FW], op=Alu.mult,
                    )
                    st["w01"] = w01

                def s3():
                    # aa|bb = (w - q)|(w + q) in one tile, floored at CLAMP
                    # (covers rounding negatives + the reference's clip;
                    # degenerate/padded edges: both floored -> t = 1 -> pi/4
                    # on both meshes -> delta 0), then one in-place Ln.
                    qnm, w01 = st["qnm"], st["w01"]
                    q01 = qnm[:, 0 : 2 * FW]
                    wv = w01[:, 0 : 2 * FW]
                    ab = smp.tile([P, 4 * aw], f16, tag="ab" + sx)
                    nc.vector.tensor_tensor(
                        out=ab[:, 0 : 2 * FW], in0=wv, in1=q01, op=Alu.subtract
                    )
                    nc.vector.tensor_tensor(
                        out=ab[:, 2 * FW : 4 * FW], in0=wv, in1=q01, op=Alu.add
                    )
                    nc.vector.tensor_scalar(
                        out=ab[:, 0 : 4 * FW], in0=ab[:, 0 : 4 * FW],
                        scalar1=CLAMP, scalar2=None, op0=Alu.max,
                    )
                    nc.scalar.activation(
                        out=ab[:, 0 : 4 * FW], in_=ab[:, 0 : 4 * FW], func=AF.Ln
                    )
                    st["lab"] = ab

                def s4():
                    lab = st["lab"]
                    zv = smp.tile([P, 2 * aw], f16, tag="zv" + sx)
                    nc.vector.tensor_tensor(
                        out=zv[:, 0 : 2 * FW], in0=lab[:, 0 : 2 * FW],
                        in1=lab[:, 2 * FW : 4 * FW], op=Alu.subtract,
                    )
                    # t fits fp16: the CLAMP floor bounds t = exp(z/2) by
                    # sqrt(max(w+q)/CLAMP) ~ 2.7e4 < 65504
                    tv = smp.tile([P, 2 * aw], f16, tag="tv" + sx)
                    nc.scalar.activation(
                        out=tv[:, 0 : 2 * FW], in_=zv[:, 0 : 2 * FW],
                        func=AF.Exp, scale=0.5,
                    )
                    nc.scalar.activation(
                        out=tv[:, 0 : 2 * FW], in_=tv[:, 0 : 2 * FW], func=AF.Arctan
                    )
                    st["at"] = tv

                def s5():
                    # delta = at0 - at1; acc[:, col] = sum delta^2 (ACT accum)
                    at = st["at"]
                    d = smp.tile([P, aw], f16, tag="d" + sx)
                    nc.vector.tensor_tensor(
                        out=d[:, 0:FW], in0=at[:, 0:FW], in1=at[:, FW : 2 * FW],
                        op=Alu.subtract,
                    )
                    dd = smp.tile([P, aw], f32, tag="dd" + sx)
                    nc.scalar.activation(
                        out=dd[:, 0:FW], in_=d[:, 0:FW], func=AF.Square,
                        accum_out=acc[:, col : col + 1],
                    )

                return [s1, s2, s3, s4, s5]

            pend = None
            for t in range(T):
                if t == T - 1 and pend is not None:
                    # fold tail(T-2) into the last geometry via hooks; split
                    # tail(T-1) into edge-halves and interleave all three so
                    # every end-of-program ACT hop overlaps sibling DVE work
                    st_a = tail_stages(pend[0], pend[1])
                    holder = {}

                    def _b0(pq):
                        holder["b0"] = tail_stages(T - 1, pq, h=0)
                        holder["b1"] = tail_stages(T, pq, h=1)
                        holder["b0"][0]()

                    geo(t, hooks=[st_a[0], st_a[1], st_a[2], _b0])
                    b0, b1 = holder["b0"], holder["b1"]
                    b1[0]()
                    b0[1]()
                    st_a[3]()
                    b1[1]()
                    b0[2]()
                    st_a[4]()
                    b1[2]()
                    b0[3]()
                    b1[3]()
                    b0[4]()
                    # writeback of all finished accumulator columns overlaps
                    # the final half-tail; only the last column trails it
                    nc.sync.dma_start(out=out[:, 0:T], in_=acc[:, 0:T])
                    b1[4]()
                    pend = None
                else:
                    pq = geo(t)
                    if pend is not None:
                        for s in tail_stages(pend[0], pend[1]):
                            s()
                    pend = (t, pq)

            nc.sync.dma_start(out=out[:, T : T + 1], in_=acc[:, T : T + 1])

    _split_multi_waits(nc)
    return nc


def _split_multi_waits(nc: bass.Bass) -> None:
    """Two post-scheduling wait cleanups:

    1. Drop redundant waits: each engine's sequencer executes waits in
       program order and semaphore values are monotone within the kernel
       body, so a wait on (sem >= v) is a no-op if an earlier instruction
       on the same engine already waited (sem >= v') with v' >= v.  Dedup
       stops at the first DRAIN (the kernel-tail drain resets sems).
    2. Walrus accepts at most ONE sync wait per (non-drain) instruction;
       hoist extras onto injected same-engine event-semaphore instructions
       placed immediately before -- semantically identical."""
    import bass_rust

    ctr = 0
    for fn in nc.m.functions:
        for bb in fn.blocks:
            observed: dict = {}  # (engine, sem_id) -> max waited value
            dedup_on = True
            new_list = []
            for inst in bb.instructions:
                if isinstance(inst, mybir.InstDrain):
                    dedup_on = False
                si = getattr(inst, "sync_info", None)
                if si is not None and si.on_wait and dedup_on:
                    kept = []
                    for w in si.on_wait:
                        if (
                            w.sync_type == "semaphore"
                            and w.wait_mode == "sem-ge-imm"
                            and getattr(w, "wait_reg", None) is None
                        ):
                            key = (str(inst.engine), w.id)
                            if observed.get(key, -1) >= w.wait_value:
                                continue
                            observed[key] = w.wait_value
                        kept.append(w)
                    if len(kept) != len(si.on_wait):
                        si = bass_rust.SyncInfo(
                            on_wait=kept, on_update=list(si.on_update)
                        )
                        inst.sync_info = si
                if si is not None and len(si.on_wait) > 1:
                    waits = list(si.on_wait)
                    for w in waits[:-1]:
                        ev = mybir.InstEventSemaphore(name=f"I-waitsplit-{ctr}")
                        ctr += 1
                        ev.engine = inst.engine
                        ev.sync_info = bass_rust.SyncInfo(
                            on_wait=[w], on_update=[]
                        )
                        new_list.append(ev)
                    inst.sync_info = bass_rust.SyncInfo(
                        on_wait=[waits[-1]], on_update=list(si.on_update)
                    )
                new_list.append(inst)
            bb.instructions = new_list


def _get_nc() -> bass.Bass:
    if "nc" not in _CACHE:
        _CACHE["nc"] = _build_program()
    return _CACHE["nc"]


def _prep_in_maps(vert1, vert2, edge_points):
    in_maps = []
    for b in range(B):
        tbl = np.concatenate(
            [np.asarray(vert1[b], np.float32), np.asarray(vert2[b], np.float32)],
            axis=1,
        )  # [N, 6]
        ep = np.asarray(edge_points[b]).astype(np.int32)  # [E, 4]
        pad = np.zeros((EPAD, 4), np.int32)
        pad[:E] = ep
        # edge (t, p, w) = (t*P + p)*F + w; gather rows then lay out
        # plane-major: pg[t, p, (c, j, w)] with c = 3*mesh + xyz
        g = tbl.astype(np.float16)[pad.reshape(T, P, F, 4)]  # [T, P, F, 4, 6]
        pgb = np.ascontiguousarray(g.transpose(0, 1, 4, 3, 2)).reshape(T, P, 24 * F)
        in_maps.append({"pg": pgb})
    return in_maps


def _run(in_maps, **kwargs):
    nc = _get_nc()
    return run_bass_kernel_spmd(nc, in_maps, core_ids=list(range(B)), **kwargs)


def _finalize(results) -> np.ndarray:
    total = 0.0
    for rmap in results:
        total += float(np.asarray(rmap["out"], np.float64).sum())
    # angle diff = 2*(atan1 - atan2)  ->  factor 4 on the squared sums
    return np.asarray(np.float32(4.0 * total / (B * E)))


def kernel(vert1, vert2, edge_points) -> np.ndarray:
    in_maps = _prep_in_maps(vert1, vert2, edge_points)
    res = _run(in_maps)
    return _finalize(res.results)


# revision 28
# speedup vs baseline: 1.0143x; 1.0143x over previous
"""Trainium2 Bass kernel for MeshDihedralAngleLoss.

Reference computation (per batch b, per edge e with ep = edge_points[b,e] =
[v0, v1, a, b]):
    na = normalize(cross(verts[a]-verts[v0], verts[v1]-verts[v0]))
    nb = normalize(cross(verts[b]-verts[v1], verts[v0]-verts[v1]))
    angle = pi - arccos(clip(dot(na, nb), +-(1-1e-7)))
computed for vert1 and vert2; loss = mean_b mean_e (angle1-angle2)^2.

Algebra: with ea = p2-p0, eb = p1-p0, d3 = p3-p1, nb = cross(d3, -eb) = -m,
    angle = pi - arccos(-u) = arccos(u),  u = dot(na, m)/(|na||m|)
and with q = dot(na, m), w = |na||m|:
    arccos(u) = 2*atan(sqrt((w - q)/(w + q)))
              = 2*atan(exp(0.5*(ln(w - q) - ln(w + q))))     [division-free]
so angle1 - angle2 = 2*(atan(t1) - atan(t2)) and the host applies the *4
factor on the squared sums plus the global mean (the only cross-core step).
w -+ q are clamped to [1e-4, inf): this covers f32/f16 rounding pushing them
negative AND the u-clip of the reference (different floor, same measure-zero
edges), and keeps Ln's input strictly positive with no bias const needed.

Sharding: pure data parallel, core b <- mesh b (B == 8 == n_cores).

Host marshaling: the indexed gather is pure data movement, so it is done
host-side with numpy fancy indexing (same class of marshaling as index
pre-tiling): each core receives its edges' vertex coordinates pre-gathered
(fp16) into the exact plane-major SBUF layout
    pg[t, p, ((m*3 + c)*4 + j)*F + w] = verts_m[ep[e, j], c],
    e = (t*P + p)*F + w
(m = mesh 0/1, c = xyz, j = vertex slot 0..3).  The device then streams
sequential DRAM at full DMA bandwidth -- no per-edge descriptors.

Engine split (software-pipelined by one chunk so DVE never waits on ACT):
  DVE:  edge vectors, cross products, dot products (fp16, 2x mode) + glue
  ACT:  squares (split na/mm halves), sqrt, ln/exp (the division), arctan,
        final sum-accumulate
The last chunk's geometry takes the previous tail's stages as hooks placed
so its ACT ops interleave between the square halves on the ACT queue, and
the final two tails are stage-interleaved -- the end-of-program ACT chains
overlap remaining DVE work.
"""

import numpy as np

import concourse.bass as bass
import concourse.mybir as mybir
from concourse.tile import TileContext
from concourse.bass_utils import run_bass_kernel_spmd

dt = mybir.dt
Alu = mybir.AluOpType
AF = mybir.ActivationFunctionType

B, N, E = 8, 100000, 300000
P = 128
F = 586            # edges per partition per chunk
T = 4              # chunks; P*F*T = 300032 >= E (32 zero-padded edges)
EPAD = P * F * T
F2 = F // 2        # width of the half-tails pipelined at the kernel end
CLAMP = 1e-4       # fp16-normal floor for w -+ q (see module docstring)

_CACHE: dict = {}


def _build_program() -> bass.Bass:
    nc = bass.Bass(trn_type="TRN2")
    f32 = dt.float32
    f16 = dt.float16
    pg = nc.dram_tensor("pg", [T, P, 24 * F], f16, kind="ExternalInput")
    out = nc.dram_tensor("out", [P, T + 1], f32, kind="ExternalOutput")

    with TileContext(nc) as tc:
        with (
            tc.tile_pool(name="accp", bufs=1) as accp,
            tc.tile_pool(name="iop", bufs=2) as iop,
            tc.tile_pool(name="wkp", bufs=1) as wkp,
            tc.tile_pool(name="pip", bufs=2) as pip,
            tc.tile_pool(name="smp", bufs=1) as smp,
        ):
            acc = accp.tile([P, T + 1], f32)

            def geo(t, hooks=()):
                """DVE geometry for chunk t; returns pq = [na*mm | na^2|mm^2].
                hooks: up to 4 callables emitted between geometry phases (the
                last chunk interleaves the previous tail's stages here so its
                ACT ops slot between this chunk's square halves)."""
                hooks = list(hooks) + [None] * 4
                gall = iop.tile([P, 24 * F], f16, tag="gall")
                if t == 0:
                    # split the cold-start DMA, first slice = exactly the
                    # columns the first subtraction reads (mesh-0 planes y,z;
                    # Tile deps are per-slice and DMA queues drain in issue
                    # order, so compute starts ~1/3 into the transfer)
                    nc.sync.dma_start(
                        out=gall[:, 4 * F : 12 * F], in_=pg[t, :, 4 * F : 12 * F]
                    )
                    nc.sync.dma_start(out=gall[:, 0 : 4 * F], in_=pg[t, :, 0 : 4 * F])
                    nc.sync.dma_start(out=gall[:, 12 * F :], in_=pg[t, :, 12 * F :])
                else:
                    nc.sync.dma_start(out=gall[:], in_=pg[t, :, :])
                # planes: [m:2][c:3][j:4][w:F]
                gv = gall[:].rearrange("p (m c j w) -> p m c j w", m=2, c=3, j=4)

                # Edge vectors in 4-plane buffers (y,z,x,y per mesh) -- only
                # the planes the cross products read; component rotation is a
                # plain +-F offset shift.
                ea4 = wkp.tile([P, 8 * F], f16, tag="ea4")
                eb4 = wkp.tile([P, 8 * F], f16, tag="eb4")
                d34 = wkp.tile([P, 8 * F], f16, tag="d34")
                mesh_splits = ((0, 2),) if t != 0 else ((0, 1), (1, 2))
                for m0, m1 in mesh_splits:
                    for buf, js, jb in ((ea4, 2, 0), (eb4, 1, 0), (d34, 3, 1)):
                        bv = buf[:].rearrange("p (m k w) -> p m k w", m=2, k=4)
                        nc.vector.tensor_tensor(
                            out=bv[:, m0:m1, 0:2, :],
                            in0=gv[:, m0:m1, 1:3, js, :],
                            in1=gv[:, m0:m1, 1:3, jb, :],
                            op=Alu.subtract,
                        )
                        nc.vector.tensor_tensor(
                            out=bv[:, m0:m1, 2:4, :],
                            in0=gv[:, m0:m1, 0:2, js, :],
                            in1=gv[:, m0:m1, 0:2, jb, :],
                            op=Alu.subtract,
                        )
                if hooks[0]:
                    hooks[0]()

                # na = ea x eb ; mm = d3 x eb  (plane-major per mesh), packed
                # namm = [na_m0 | na_m1 | mm_m0 | mm_m1] x 3F
                namm = pip.tile([P, 12 * F], f16, tag="namm")
                pq = pip.tile([P, 18 * F], f16, tag="pq")
                tmp6 = wkp.tile([P, 6 * F], f16, tag="t6")
                t6v = tmp6[:].rearrange("p (m k w) -> p m k w", m=2, k=3)
                b4 = eb4[:].rearrange("p (m k w) -> p m k w", m=2, k=4)
                for half, (dst0, av) in enumerate(((0, ea4), (6 * F, d34))):
                    dstv = namm[:, dst0 : dst0 + 6 * F].rearrange(
                        "p (m k w) -> p m k w", m=2, k=3
                    )
                    a4 = av[:].rearrange("p (m k w) -> p m k w", m=2, k=4)
                    nc.vector.tensor_tensor(
                        out=dstv, in0=a4[:, :, 0:3, :], in1=b4[:, :, 1:4, :],
                        op=Alu.mult,
                    )
                    nc.vector.tensor_tensor(
                        out=t6v, in0=a4[:, :, 1:4, :], in1=b4[:, :, 0:3, :],
                        op=Alu.mult,
                    )
                    nc.vector.tensor_tensor(
                        out=dstv, in0=dstv, in1=t6v, op=Alu.subtract
                    )
                    # this half's squares on ACT, overlapping the next DVE work
                    nc.scalar.activation(
                        out=pq[:, 6 * F + dst0 : 12 * F + dst0],
                        in_=namm[:, dst0 : dst0 + 6 * F], func=AF.Square,
                    )
                    if hooks[1 + half]:
                        hooks[1 + half]()

                nc.vector.tensor_tensor(
                    out=pq[:, 0 : 6 * F], in0=namm[:, 0 : 6 * F],
                    in1=namm[:, 6 * F : 12 * F], op=Alu.mult,
                )
                if hooks[3]:
                    hooks[3](pq)
                return pq

            def tail_stages(col, pq, h=None):
                """Chunk tail as fine stages (DVE ops + the ACT hop they
                feed), emitted after the NEXT chunk's geometry.  h=0/1
                processes one edge-half (width F2) -- the kernel-end tails
                are split this way and interleaved so their ACT chains
                pipeline against the sibling half's DVE work.  Half 0 reuses
                the full-size tile tags (sliced); half 1 gets compact tags."""
                st = {}
                FW = F if h is None else F2   # logical width
                aw = F2 if h == 1 else F      # allocation width (tag sizing)
                sx = "1" if h == 1 else ""
                w0 = F2 if h == 1 else 0      # pq w-offset

                def s1():
                    # plane sums: [q | na2 | m2] for both meshes in one pair
                    pqv = pq[:].rearrange("p (g k w) -> p g k w", g=6, k=3)
                    qnm = smp.tile([P, 6 * aw], f16, tag="qnm" + sx)
                    qv = qnm[:, 0 : 6 * FW].rearrange("p (g w) -> p g w", g=6)
                    nc.vector.tensor_tensor(
                        out=qv, in0=pqv[:, :, 0, w0 : w0 + FW],
                        in1=pqv[:, :, 1, w0 : w0 + FW], op=Alu.add,
                    )
                    nc.vector.tensor_tensor(
                        out=qv, in0=qv, in1=pqv[:, :, 2, w0 : w0 + FW], op=Alu.add
                    )
                    st["qnm"] = qnm
                    # sqrt of the norms (ACT) queued right behind the sums
                    sn = smp.tile([P, 4 * aw], f16, tag="sn" + sx)
                    nc.scalar.activation(
                        out=sn[:, 0 : 4 * FW], in_=qnm[:, 2 * FW : 6 * FW],
                        func=AF.Sqrt,
                    )
                    st["sn"] = sn

                def s2():
                    # w = |na||m| = sqrt(na2)*sqrt(m2), fp16 2x
                    sn = st["sn"]
                    w01 = smp.tile([P, 2 * aw], f16, tag="w01" + sx)
                    nc.vector.tensor_tensor(
                        out=w01[:, 0 : 2 * FW], in0=sn[:, 0 : 2 * FW],
                        in1=sn[:, 2 * FW : 4 * FW], op=Alu.mult,
                    )
                    st["w01"] = w01

                def s3():
                    # aa|bb = (w - q)|(w + q) in one tile, floored at CLAMP
                    # (covers rounding negatives + the reference's clip;
                    # degenerate/padded edges: both floored -> t = 1 -> pi/4
                    # on both meshes -> delta 0), then one in-place Ln.
                    qnm, w01 = st["qnm"], st["w01"]
                    q01 = qnm[:, 0 : 2 * FW]
                    wv = w01[:, 0 : 2 * FW]
                    ab = smp.tile([P, 4 * aw], f16, tag="ab" + sx)
                    nc.vector.tensor_tensor(
                        out=ab[:, 0 : 2 * FW], in0=wv, in1=q01, op=Alu.subtract
                    )
                    nc.vector.tensor_tensor(
                        out=ab[:, 2 * FW : 4 * FW], in0=wv, in1=q01, op=Alu.add
                    )
                    nc.vector.tensor_scalar(
                        out=ab[:, 0 : 4 * FW], in0=ab[:, 0 : 4 * FW],
                        scalar1=CLAMP, scalar2=None, op0=Alu.max,
                    )
                    nc.scalar.activation(
                        out=ab[:, 0 : 4 * FW], in_=ab[:, 0 : 4 * FW], func=AF.Ln
                    )
                    st["lab"] = ab

                def s4():
                    lab = st["lab"]
                    zv = smp.tile([P, 2 * aw], f16, tag="zv" + sx)
                    nc.vector.tensor_tensor(
                        out=zv[:, 0 : 2 * FW], in0=lab[:, 0 : 2 * FW],
                        in1=lab[:, 2 * FW : 4 * FW], op=Alu.subtract,
                    )
                    # Exp out in f32: t can overflow fp16 for near-pi angles
                    tv = smp.tile([P, 2 * aw], f32, tag="tv" + sx)
                    nc.scalar.activation(
                        out=tv[:, 0 : 2 * FW], in_=zv[:, 0 : 2 * FW],
                        func=AF.Exp, scale=0.5,
                    )
                    nc.scalar.activation(
                        out=tv[:, 0 : 2 * FW], in_=tv[:, 0 : 2 * FW], func=AF.Arctan
                    )
                    st["at"] = tv

                def s5():
                    # delta = at0 - at1; acc[:, col] = sum delta^2 (ACT accum)
                    at = st["at"]
                    d = smp.tile([P, aw], f32, tag="d" + sx)
                    nc.vector.tensor_tensor(
                        out=d[:, 0:FW], in0=at[:, 0:FW], in1=at[:, FW : 2 * FW],
                        op=Alu.subtract,
                    )
                    dd = smp.tile([P, aw], f32, tag="dd" + sx)
                    nc.scalar.activation(
                        out=dd[:, 0:FW], in_=d[:, 0:FW], func=AF.Square,
                        accum_out=acc[:, col : col + 1],
                    )

                return [s1, s2, s3, s4, s5]

            pend = None
            for t in range(T):
                if t == T - 1 and pend is not None:
                    # fold tail(T-2) into the last geometry via hooks; split
                    # tail(T-1) into edge-halves and interleave all three so
                    # every end-of-program ACT hop overlaps sibling DVE work
                    st_a = tail_stages(pend[0], pend[1])
                    holder = {}

                    def _b0(pq):
                        holder["b0"] = tail_stages(T - 1, pq, h=0)
                        holder["b1"] = tail_stages(T, pq, h=1)
                        holder["b0"][0]()

                    geo(t, hooks=[st_a[0], st_a[1], st_a[2], _b0])
                    b0, b1 = holder["b0"], holder["b1"]
                    b1[0]()
                    b0[1]()
                    st_a[3]()
                    b1[1]()
                    b0[2]()
                    st_a[4]()
                    b1[2]()
                    b0[3]()
                    b1[3]()
                    b0[4]()
                    b1[4]()
                    pend = None
                else:
                    pq = geo(t)
                    if pend is not None:
                        for s in tail_stages(pend[0], pend[1]):
                            s()
                    pend = (t, pq)

            nc.sync.dma_start(out=out[:, :], in_=acc[:])

    _split_multi_waits(nc)
    return nc


def _split_multi_waits(nc: bass.Bass) -> None:
    """Two post-scheduling wait cleanups:

    1. Drop redundant waits: each engine's sequencer executes waits in
       program order and semaphore values are monotone within the kernel
       body, so a wait on (sem >= v) is a no-op if an earlier instruction
       on the same engine already waited (sem >= v') with v' >= v.  Dedup
       stops at the first DRAIN (the kernel-tail drain resets sems).
    2. Walrus accepts at most ONE sync wait per (non-drain) instruction;
       hoist extras onto injected same-engine event-semaphore instructions
       placed immediately before -- semantically identical."""
    import bass_rust

    ctr = 0
    for fn in nc.m.functions:
        for bb in fn.blocks:
            observed: dict = {}  # (engine, sem_id) -> max waited value
            dedup_on = True
            new_list = []
            for inst in bb.instructions:
                if isinstance(inst, mybir.InstDrain):
                    dedup_on = False
                si = getattr(inst, "sync_info", None)
                if si is not None and si.on_wait and dedup_on:
                    kept = []
                    for w in si.on_wait:
                        if (
                            w.sync_type == "semaphore"
                            and w.wait_mode == "sem-ge-imm"
                            and getattr(w, "wait_reg", None) is None
                        ):
                            key = (str(inst.engine), w.id)
                            if observed.get(key, -1) >= w.wait_value:
                                continue
                            observed[key] = w.wait_value
                        kept.append(w)
                    if len(kept) != len(si.on_wait):
                        si = bass_rust.SyncInfo(
                            on_wait=kept, on_update=list(si.on_update)
                        )
                        inst.sync_info = si
                if si is not None and len(si.on_wait) > 1:
                    waits = list(si.on_wait)
                    for w in waits[:-1]:
                        ev = mybir.InstEventSemaphore(name=f"I-waitsplit-{ctr}")
                        ctr += 1
                        ev.engine = inst.engine
                        ev.sync_info = bass_rust.SyncInfo(
                            on_wait=[w], on_update=[]
                        )
                        new_list.append(ev)
                    inst.sync_info = bass_rust.SyncInfo(
                        on_wait=[waits[-1]], on_update=list(si.on_update)
                    )
                new_list.append(inst)
            bb.instructions = new_list


def _get_nc() -> bass.Bass:
    if "nc" not in _CACHE:
        _CACHE["nc"] = _build_program()
    return _CACHE["nc"]


def _prep_in_maps(vert1, vert2, edge_points):
    in_maps = []
    for b in range(B):
        tbl = np.concatenate(
            [np.asarray(vert1[b], np.float32), np.asarray(vert2[b], np.float32)],
            axis=1,
        )  # [N, 6]
        ep = np.asarray(edge_points[b]).astype(np.int32)  # [E, 4]
        pad = np.zeros((EPAD, 4), np.int32)
        pad[:E] = ep
        # edge (t, p, w) = (t*P + p)*F + w; gather rows then lay out
        # plane-major: pg[t, p, (c, j, w)] with c = 3*mesh + xyz
        g = tbl.astype(np.float16)[pad.reshape(T, P, F, 4)]  # [T, P, F, 4, 6]
        pgb = np.ascontiguousarray(g.transpose(0, 1, 4, 3, 2)).reshape(T, P, 24 * F)
        in_maps.append({"pg": pgb})
    return in_maps


def _run(in_maps, **kwargs):
    nc = _get_nc()
    return run_bass_kernel_spmd(nc, in_maps, core_ids=list(range(B)), **kwargs)


def _finalize(results) -> np.ndarray:
    total = 0.0
    for rmap in results:
        total += float(np.asarray(rmap["out"], np.float64).sum())
    # angle diff = 2*(atan1 - atan2)  ->  factor 4 on the squared sums
    return np.asarray(np.float32(4.0 * total / (B * E)))


def kernel(vert1, vert2, edge_points) -> np.ndarray:
    in_maps = _prep_in_maps(vert1, vert2, edge_points)
    res = _run(in_maps)
    return _finalize(res.results)
